# revision 24
# baseline (speedup 1.0000x reference)
"""Chamfer distance loss kernel for Trainium2 (8 NeuronCores, data-parallel over batch).

Strategy:
  - B=16 batches sharded 2 per core across 8 cores.
  - Per batch, d2[n, m] = ||p_n||^2 + ||g_m||^2 - 2 p_n . g_m is computed
    directly by a K=9 augmented matmul on the PE:
       lhsT rows: [px^2, py^2, pz^2, 1, 1, 1, px, py, pz]       (predict)
       rhs  rows: [1, 1, 1, gx^2, gy^2, gz^2, -2gx, -2gy, -2gz] (gt)
    Operands are split on the host into fp16 hi + fp16 lo halves and the
    three compensation products are folded into a single K=27 fp16 matmul:
    lhsT rows [ph; pl; ph] against rhs rows [gh; gh; gl] accumulate
    ph*gh + pl*gh + ph*gl, giving ~fp32 accuracy at one fp16 matmul's
    cost (the stream of 512 rhs columns dominates; K is nearly free).
    The four 512-wide m-chunks of each 128-row n-chunk run concurrently
    on the four 32-row PE groups (tile_position row packing), writing
    four different PSUM banks.
  - Each [128, 2048] PSUM chunk leaves PSUM exactly once, via a ScalarE
    copy to fp16 SBUF.
  - z2 (min over gt points, free dim) via one DVE tensor_scalar accum-min
    (4x fp16 mode) per chunk.
  - z (min over predict points, partition dim): DVE running elementwise
    min accumulator (fp16 2x mode); the [128, 2048] accumulator is DMA'd
    out and the final 128-way partition min is done on the host.
  - Host takes sqrt of the min-d2 values and sums into the scalar loss.
"""

import numpy as np

import concourse.bass as bass
import concourse.tile as tile
from concourse import bacc, bass_utils, mybir

B = 16  # total batches
NCORES = 8
BPC = B // NCORES  # batches per core
N = 2048  # points per cloud
NCHUNK = 16  # chunks of 128 predict points
MCHUNK = 4  # chunks of 512 gt points

F32 = mybir.dt.float32
FP16 = mybir.dt.float16
MIN = mybir.AluOpType.min
AXX = mybir.AxisListType.X
FP16_BIG = 60000.0  # min-identity init (all d2 values are << this)


def _build_program():
    nc = bacc.Bacc("TRN2", target_bir_lowering=False, debug=False)
    # 27 K-rows: p-side [ph; pl; ph] in columns 0:N, g-side [gh; gh; gl]
    # in columns N:2N
    pg_in = nc.dram_tensor("pg_in", (BPC, 27, 2 * N), FP16, kind="ExternalInput")
    # z2 mins (per-predict-point min d2): [b, p, i] is predict point i*128+p
    mins = nc.dram_tensor("mins", (BPC, 128, NCHUNK), F32, kind="ExternalOutput")
    # z accumulator (per (p, m): min d2 over predict points n = i*128+p);
    # host finishes the 128-way min over p
    accs = nc.dram_tensor("accs", (BPC, 128, N), FP16, kind="ExternalOutput")

    with tile.TileContext(nc) as tc:
        with (
            tc.tile_pool(name="aug", bufs=2) as aug_pool,
            tc.tile_pool(name="d2p", bufs=2, space="PSUM") as psum_pool,
            tc.tile_pool(name="cpp", bufs=8) as cp_pool,
            tc.tile_pool(name="accp", bufs=2) as acc_pool,
            tc.tile_pool(name="outp", bufs=2) as out_pool,
        ):
            for b in range(BPC):
                # operand replicas at partition bases 0/32/64/96 so the four
                # m-chunk matmuls of a chunk run on distinct PE row groups
                aug = aug_pool.tile([128, 2 * N], FP16, tag="aug")
                for g in range(MCHUNK):
                    nc.sync.dma_start(aug[32 * g : 32 * g + 27, :], pg_in[b])

                z2t = out_pool.tile([128, NCHUNK], F32, tag="z2")
                acc1 = acc_pool.tile([128, N], FP16, tag="acc1")
                nc.gpsimd.memset(acc1[:], FP16_BIG)

                for i in range(NCHUNK):
                    d2 = psum_pool.tile([128, N], F32, tag="d2")
                    for j in range(MCHUNK):
                        base = 32 * j
                        nc.tensor.matmul(
                            d2[:, j * 512 : (j + 1) * 512],
                            aug[base : base + 27, i * 128 : (i + 1) * 128],
                            aug[base : base + 27, N + j * 512 : N + (j + 1) * 512],
                            start=True,
                            stop=True,
                            tile_position=(base, 0),
                        )
                    # single PSUM egress per element: ACT copies to fp16
                    # SBUF; DVE then does a 4x-mode accum-min tensor_scalar
                    # for z2 (the in-place elementwise result is a no-op,
                    # only accum_out matters).
                    cp = cp_pool.tile([128, N], FP16, tag="cp")
                    nc.scalar.copy(cp[:], d2[:])
                    nc.vector.tensor_scalar(
                        cp[:], cp[:], FP16_BIG, None,
                        op0=MIN, op1=MIN, accum_out=z2t[:, i : i + 1],
                    )
                    # z-path running min (fp16 2x)
                    nc.vector.tensor_tensor(acc1[:], cp[:], acc1[:], op=MIN)

                nc.sync.dma_start(accs[b], acc1[:])
                nc.sync.dma_start(mins[b], z2t[:])
    nc.compile()
    return nc


_NC_CACHE = None


def _get_nc():
    global _NC_CACHE
    if _NC_CACHE is None:
        _NC_CACHE = _build_program()
    return _NC_CACHE


def _augment(predict_pc, gt_pc):
    """Host-side marshaling into the packed K=9 augmented matmul operand,
    split into fp16 hi + lo halves: x = hi + lo with hi = fp16(x)."""
    ones = np.ones_like(predict_pc)  # [B, 3, N]
    paug = np.concatenate([predict_pc * predict_pc, ones, predict_pc], axis=1)
    gaug = np.concatenate([ones, gt_pc * gt_pc, -2.0 * gt_pc], axis=1)
    ph = paug.astype(np.float16)
    pl = (paug - ph.astype(np.float32)).astype(np.float16)
    gh = gaug.astype(np.float16)
    gl = (gaug - gh.astype(np.float32)).astype(np.float16)
    pside = np.concatenate([ph, pl, ph], axis=1)  # [B, 27, N]
    gside = np.concatenate([gh, gh, gl], axis=1)  # [B, 27, N]
    return np.ascontiguousarray(np.concatenate([pside, gside], axis=2))  # [B, 27, 2N]


def kernel(predict_pc, gt_pc):
    predict_pc = np.ascontiguousarray(np.asarray(predict_pc, dtype=np.float32))
    gt_pc = np.ascontiguousarray(np.asarray(gt_pc, dtype=np.float32))
    pg = _augment(predict_pc, gt_pc)
    nc = _get_nc()
    in_maps = [
        {"pg_in": np.ascontiguousarray(pg[c * BPC : (c + 1) * BPC])}
        for c in range(NCORES)
    ]
    res = bass_utils.run_bass_kernel_spmd(nc, in_maps, core_ids=list(range(NCORES)))
    total = 0.0
    for c in range(NCORES):
        m = np.asarray(res.results[c]["mins"], dtype=np.float64)  # [BPC, 128, 16]
        total += np.sqrt(np.maximum(m, 0.0)).sum()
        acc = np.asarray(res.results[c]["accs"], dtype=np.float32)  # [BPC, 128, N]
        z = acc.min(axis=1)  # final partition min on host
        total += np.sqrt(np.maximum(z, 0.0), dtype=np.float64).sum()
    return np.float32(total / (B * N))


# revision 26
# speedup vs baseline: 1.0086x; 1.0086x over previous
"""Chamfer distance loss kernel for Trainium2 (8 NeuronCores, data-parallel over batch).

Strategy:
  - B=16 batches sharded 2 per core across 8 cores.
  - Per batch, d2[n, m] = ||p_n||^2 + ||g_m||^2 - 2 p_n . g_m is computed
    directly by a K=9 augmented matmul on the PE:
       lhsT rows: [px^2, py^2, pz^2, 1, 1, 1, px, py, pz]       (predict)
       rhs  rows: [1, 1, 1, gx^2, gy^2, gz^2, -2gx, -2gy, -2gz] (gt)
    Operands are split on the host into fp16 hi + fp16 lo halves and the
    three compensation products are folded into a single K=27 fp16 matmul:
    lhsT rows [ph; pl; ph] against rhs rows [gh; gh; gl] accumulate
    ph*gh + pl*gh + ph*gl, giving ~fp32 accuracy at one fp16 matmul's
    cost (the stream of 512 rhs columns dominates; K is nearly free).
    The four 512-wide m-chunks of each 128-row n-chunk run concurrently
    on the four 32-row PE groups (tile_position row packing), writing
    four different PSUM banks.
  - Each [128, 2048] PSUM chunk leaves PSUM exactly once, via a ScalarE
    copy to fp16 SBUF.
  - z2 (min over gt points, free dim) via one DVE tensor_scalar accum-min
    (4x fp16 mode) per chunk.
  - z (min over predict points, partition dim): DVE running elementwise
    min accumulator (fp16 2x mode); the [128, 2048] accumulator is DMA'd
    out and the final 128-way partition min is done on the host.
  - Host takes sqrt of the min-d2 values and sums into the scalar loss.
"""

import numpy as np

import concourse.bass as bass
import concourse.tile as tile
from concourse import bacc, bass_utils, mybir

B = 16  # total batches
NCORES = 8
BPC = B // NCORES  # batches per core
N = 2048  # points per cloud
NCHUNK = 16  # chunks of 128 predict points
MCHUNK = 4  # chunks of 512 gt points

F32 = mybir.dt.float32
FP16 = mybir.dt.float16
MIN = mybir.AluOpType.min
AXX = mybir.AxisListType.X
FP16_BIG = 60000.0  # min-identity init (all d2 values are << this)


def _build_program():
    nc = bacc.Bacc("TRN2", target_bir_lowering=False, debug=False)
    # 27 K-rows: p-side [ph; pl; ph] in columns 0:N, g-side [gh; gh; gl]
    # in columns N:2N
    pg_in = nc.dram_tensor("pg_in", (BPC, 27, 2 * N), FP16, kind="ExternalInput")
    # z2 mins (per-predict-point min d2): [b, p, i] is predict point i*128+p
    mins = nc.dram_tensor("mins", (BPC, 128, NCHUNK), F32, kind="ExternalOutput")
    # z accumulator (per (p, m): min d2 over predict points n = i*128+p);
    # host finishes the 128-way min over p
    accs = nc.dram_tensor("accs", (BPC, 128, N), FP16, kind="ExternalOutput")

    with tile.TileContext(nc) as tc:
        with (
            tc.tile_pool(name="aug", bufs=2) as aug_pool,
            tc.tile_pool(name="d2p", bufs=2, space="PSUM") as psum_pool,
            tc.tile_pool(name="cpp", bufs=8) as cp_pool,
            tc.tile_pool(name="junkp", bufs=2) as junk_pool,
            tc.tile_pool(name="accp", bufs=2) as acc_pool,
            tc.tile_pool(name="outp", bufs=2) as out_pool,
        ):
            for b in range(BPC):
                # operand replicas at partition bases 0/32/64/96 so the four
                # m-chunk matmuls of a chunk run on distinct PE row groups
                aug = aug_pool.tile([128, 2 * N], FP16, tag="aug")
                for g in range(MCHUNK):
                    nc.sync.dma_start(aug[32 * g : 32 * g + 27, :], pg_in[b])

                z2t = out_pool.tile([128, NCHUNK], F32, tag="z2")
                acc1 = acc_pool.tile([128, N], FP16, tag="acc1")
                nc.gpsimd.memset(acc1[:], FP16_BIG)

                for i in range(NCHUNK):
                    d2 = psum_pool.tile([128, N], F32, tag="d2")
                    for j in range(MCHUNK):
                        base = 32 * j
                        nc.tensor.matmul(
                            d2[:, j * 512 : (j + 1) * 512],
                            aug[base : base + 27, i * 128 : (i + 1) * 128],
                            aug[base : base + 27, N + j * 512 : N + (j + 1) * 512],
                            start=True,
                            stop=True,
                            tile_position=(base, 0),
                        )
                    # single PSUM egress per element: ACT copies to fp16
                    # SBUF; DVE then does a 4x-mode accum-min tensor_scalar
                    # for z2 (elementwise result is discarded into a scratch
                    # tile so the z-path TT below doesn't false-depend on it)
                    cp = cp_pool.tile([128, N], FP16, tag="cp")
                    nc.scalar.copy(cp[:], d2[:])
                    # z-path running min (fp16 2x)
                    nc.vector.tensor_tensor(acc1[:], cp[:], acc1[:], op=MIN)
                    junk = junk_pool.tile([128, N], FP16, tag="junk")
                    nc.vector.tensor_scalar(
                        junk[:], cp[:], FP16_BIG, None,
                        op0=MIN, op1=MIN, accum_out=z2t[:, i : i + 1],
                    )

                nc.sync.dma_start(accs[b], acc1[:])
                nc.sync.dma_start(mins[b], z2t[:])
    nc.compile()
    return nc


_NC_CACHE = None


def _get_nc():
    global _NC_CACHE
    if _NC_CACHE is None:
        _NC_CACHE = _build_program()
    return _NC_CACHE


def _augment(predict_pc, gt_pc):
    """Host-side marshaling into the packed K=9 augmented matmul operand,
    split into fp16 hi + lo halves: x = hi + lo with hi = fp16(x)."""
    ones = np.ones_like(predict_pc)  # [B, 3, N]
    paug = np.concatenate([predict_pc * predict_pc, ones, predict_pc], axis=1)
    gaug = np.concatenate([ones, gt_pc * gt_pc, -2.0 * gt_pc], axis=1)
    ph = paug.astype(np.float16)
    pl = (paug - ph.astype(np.float32)).astype(np.float16)
    gh = gaug.astype(np.float16)
    gl = (gaug - gh.astype(np.float32)).astype(np.float16)
    pside = np.concatenate([ph, pl, ph], axis=1)  # [B, 27, N]
    gside = np.concatenate([gh, gh, gl], axis=1)  # [B, 27, N]
    return np.ascontiguousarray(np.concatenate([pside, gside], axis=2))  # [B, 27, 2N]


def kernel(predict_pc, gt_pc):
    predict_pc = np.ascontiguousarray(np.asarray(predict_pc, dtype=np.float32))
    gt_pc = np.ascontiguousarray(np.asarray(gt_pc, dtype=np.float32))
    pg = _augment(predict_pc, gt_pc)
    nc = _get_nc()
    in_maps = [
        {"pg_in": np.ascontiguousarray(pg[c * BPC : (c + 1) * BPC])}
        for c in range(NCORES)
    ]
    res = bass_utils.run_bass_kernel_spmd(nc, in_maps, core_ids=list(range(NCORES)))
    total = 0.0
    for c in range(NCORES):
        m = np.asarray(res.results[c]["mins"], dtype=np.float64)  # [BPC, 128, 16]
        total += np.sqrt(np.maximum(m, 0.0)).sum()
        acc = np.asarray(res.results[c]["accs"], dtype=np.float32)  # [BPC, 128, N]
        z = acc.min(axis=1)  # final partition min on host
        total += np.sqrt(np.maximum(z, 0.0), dtype=np.float64).sum()
    return np.float32(total / (B * N))


# revision 29
# speedup vs baseline: 1.0173x; 1.0087x over previous
"""Chamfer distance loss kernel for Trainium2 (8 NeuronCores, data-parallel over batch).

Strategy:
  - B=16 batches sharded 2 per core across 8 cores.
  - Per batch, d2[n, m] = ||p_n||^2 + ||g_m||^2 - 2 p_n . g_m is computed
    directly by a K=9 augmented matmul on the PE:
       lhsT rows: [px^2, py^2, pz^2, 1, 1, 1, px, py, pz]       (predict)
       rhs  rows: [1, 1, 1, gx^2, gy^2, gz^2, -2gx, -2gy, -2gz] (gt)
    Operands are split on the host into fp16 hi + fp16 lo halves and the
    three compensation products are folded into a single K=27 fp16 matmul:
    lhsT rows [ph; pl; ph] against rhs rows [gh; gh; gl] accumulate
    ph*gh + pl*gh + ph*gl, giving ~fp32 accuracy at one fp16 matmul's
    cost (the stream of 512 rhs columns dominates; K is nearly free).
    The four 512-wide m-chunks of each 128-row n-chunk run concurrently
    on the four 32-row PE groups (tile_position row packing), writing
    four different PSUM banks.
  - Each [128, 2048] PSUM chunk leaves PSUM exactly once, via a ScalarE
    copy to fp16 SBUF.
  - z2 (min over gt points, free dim) via one DVE tensor_scalar accum-min
    (4x fp16 mode) per chunk.
  - z (min over predict points, partition dim): DVE running elementwise
    min accumulator (fp16 2x mode); the [128, 2048] accumulator is DMA'd
    out and the final 128-way partition min is done on the host.
  - Host takes sqrt of the min-d2 values and sums into the scalar loss.
"""

import numpy as np

import concourse.bass as bass
import concourse.tile as tile
from concourse import bacc, bass_utils, mybir

B = 16  # total batches
NCORES = 8
BPC = B // NCORES  # batches per core
N = 2048  # points per cloud
NCHUNK = 16  # chunks of 128 predict points
MCHUNK = 4  # chunks of 512 gt points

F32 = mybir.dt.float32
FP16 = mybir.dt.float16
MIN = mybir.AluOpType.min
AXX = mybir.AxisListType.X
FP16_BIG = 60000.0  # min-identity init (all d2 values are << this)


def _build_program():
    nc = bacc.Bacc("TRN2", target_bir_lowering=False, debug=False)
    # 27 K-rows: p-side [ph; pl; ph] in columns 0:N, g-side [gh; gh; gl]
    # in columns N:2N
    pg_in = nc.dram_tensor("pg_in", (BPC, 27, 2 * N), FP16, kind="ExternalInput")
    # z2 mins (per-predict-point min d2): [b, p, i] is predict point i*128+p
    mins = nc.dram_tensor("mins", (BPC, 128, NCHUNK), F32, kind="ExternalOutput")
    # z accumulator (per (p, m): min d2 over predict points n = i*128+p);
    # host finishes the 128-way min over p
    accs = nc.dram_tensor("accs", (BPC, 128, N), FP16, kind="ExternalOutput")

    with tile.TileContext(nc) as tc:
        with (
            tc.tile_pool(name="aug", bufs=2) as aug_pool,
            tc.tile_pool(name="d2p", bufs=2, space="PSUM") as psum_pool,
            tc.tile_pool(name="cpp", bufs=8) as cp_pool,
            tc.tile_pool(name="junkp", bufs=2) as junk_pool,
            tc.tile_pool(name="accp", bufs=2) as acc_pool,
            tc.tile_pool(name="outp", bufs=2) as out_pool,
        ):
            for b in range(BPC):
                # operand replicas at partition bases 0/32/64/96 so the four
                # m-chunk matmuls of a chunk run on distinct PE row groups
                aug = aug_pool.tile([128, 2 * N], FP16, tag="aug")
                for g in range(MCHUNK):
                    nc.sync.dma_start(aug[32 * g : 32 * g + 27, :], pg_in[b])

                z2t = out_pool.tile([128, NCHUNK], F32, tag="z2")
                acc1 = acc_pool.tile([128, N], FP16, tag="acc1")
                nc.gpsimd.memset(acc1[:], FP16_BIG)

                for i in range(NCHUNK):
                    d2 = psum_pool.tile([128, N], F32, tag="d2")
                    for j in range(MCHUNK):
                        base = 32 * j
                        nc.tensor.matmul(
                            d2[:, j * 512 : (j + 1) * 512],
                            aug[base : base + 27, i * 128 : (i + 1) * 128],
                            aug[base : base + 27, N + j * 512 : N + (j + 1) * 512],
                            start=True,
                            stop=True,
                            tile_position=(base, 0),
                        )
                    # single PSUM egress per element: ACT copies to fp16
                    # SBUF; DVE then does a 4x-mode accum-min tensor_scalar
                    # for z2 (elementwise result is discarded into a scratch
                    # tile so the z-path TT below doesn't false-depend on it)
                    cp = cp_pool.tile([128, N], FP16, tag="cp")
                    nc.scalar.copy(cp[:], d2[:])
                    # z-path running min (fp16 2x). The last chunk is split
                    # into halves so the accumulator DMA-out overlaps the
                    # second half's min (shorter pipeline tail).
                    if i == NCHUNK - 1:
                        h = N // 2
                        nc.vector.tensor_tensor(
                            acc1[:, 0:h], cp[:, 0:h], acc1[:, 0:h], op=MIN
                        )
                        nc.sync.dma_start(accs[b][:, 0:h], acc1[:, 0:h])
                        nc.vector.tensor_tensor(
                            acc1[:, h:N], cp[:, h:N], acc1[:, h:N], op=MIN
                        )
                        nc.sync.dma_start(accs[b][:, h:N], acc1[:, h:N])
                    else:
                        nc.vector.tensor_tensor(acc1[:], cp[:], acc1[:], op=MIN)
                    junk = junk_pool.tile([128, N], FP16, tag="junk")
                    nc.vector.tensor_scalar(
                        junk[:], cp[:], FP16_BIG, None,
                        op0=MIN, op1=MIN, accum_out=z2t[:, i : i + 1],
                    )

                nc.sync.dma_start(mins[b], z2t[:])
    nc.compile()
    return nc


_NC_CACHE = None


def _get_nc():
    global _NC_CACHE
    if _NC_CACHE is None:
        _NC_CACHE = _build_program()
    return _NC_CACHE


def _augment(predict_pc, gt_pc):
    """Host-side marshaling into the packed K=9 augmented matmul operand,
    split into fp16 hi + lo halves: x = hi + lo with hi = fp16(x)."""
    ones = np.ones_like(predict_pc)  # [B, 3, N]
    paug = np.concatenate([predict_pc * predict_pc, ones, predict_pc], axis=1)
    gaug = np.concatenate([ones, gt_pc * gt_pc, -2.0 * gt_pc], axis=1)
    ph = paug.astype(np.float16)
    pl = (paug - ph.astype(np.float32)).astype(np.float16)
    gh = gaug.astype(np.float16)
    gl = (gaug - gh.astype(np.float32)).astype(np.float16)
    pside = np.concatenate([ph, pl, ph], axis=1)  # [B, 27, N]
    gside = np.concatenate([gh, gh, gl], axis=1)  # [B, 27, N]
    return np.ascontiguousarray(np.concatenate([pside, gside], axis=2))  # [B, 27, 2N]


def kernel(predict_pc, gt_pc):
    predict_pc = np.ascontiguousarray(np.asarray(predict_pc, dtype=np.float32))
    gt_pc = np.ascontiguousarray(np.asarray(gt_pc, dtype=np.float32))
    pg = _augment(predict_pc, gt_pc)
    nc = _get_nc()
    in_maps = [
        {"pg_in": np.ascontiguousarray(pg[c * BPC : (c + 1) * BPC])}
        for c in range(NCORES)
    ]
    res = bass_utils.run_bass_kernel_spmd(nc, in_maps, core_ids=list(range(NCORES)))
    total = 0.0
    for c in range(NCORES):
        m = np.asarray(res.results[c]["mins"], dtype=np.float64)  # [BPC, 128, 16]
        total += np.sqrt(np.maximum(m, 0.0)).sum()
        acc = np.asarray(res.results[c]["accs"], dtype=np.float32)  # [BPC, 128, N]
        z = acc.min(axis=1)  # final partition min on host
        total += np.sqrt(np.maximum(z, 0.0), dtype=np.float64).sum()
    return np.float32(total / (B * N))
